# revision 28
# baseline (speedup 1.0000x reference)
"""Trainium2 Bass kernel for nn_FCN dense MLP.

Reference computation (all fp32):
    y = x                                  # [8192, 1024]
    for w in (w0, w1, w2, w3):             # w: [out, in]
        y = relu((y @ w.T) / sqrt(in))
    out = (y @ beta) / 2048                # beta: [2048, 128] -> [8192, 128]

Strategy:
  - Data-parallel: shard batch 8192 -> 8 cores x 1024 rows. No collectives.
  - Host-side prep (free, not on HW critical path):
      * fold 1/sqrt(in) into each weight, 1/H into beta
      * transpose weights to [in, out], pre-tile to [MT, 128, KT*128] so each
        per-core DMA strip is fully contiguous
      * cast x and weights to fp16 (PE upconverts to FP22, accumulates fp32;
        fp16 keeps 11 mantissa bits -> ~1e-3 relative error)
      * transpose x shard to feature-major [IN, BS]
  - On chip, activations stay feature-major [feature, batch] in SBUF so each
    layer's PSUM output tile [out_feat 128, batch 512] feeds the next layer
    directly as the moving operand (no transposes anywhere on-chip).
  - matmul: lhsT = weight tile [K=128 in-feat, M=128 out-feat] (stationary,
    fp16 -> FWL fast weight load), rhs = act tile [K=128, N=512] (moving,
    fp16 -> 1 cycle/row). PSUM fp32 accumulation over K tiles.
  - ReLU fused into the PSUM->SBUF copy (DVE / ACT alternating), output fp16.
  - Head/tail tuned from trace: 6 warm-up MMs (not 80); layer 0 runs as two
    batch-chunk phases so its matmul stream starts after ~0.25 MB of DMA
    instead of the full 2 MB x load (which is HBM-bandwidth-bound ~6us);
    readout in 4 N=256 chunks with fp16 stores pipelined under the final
    matmuls (host upcasts to fp32).
"""

import sys

if "/opt/trn_rl_repo" not in sys.path:
    sys.path.insert(0, "/opt/trn_rl_repo")

import numpy as np

B, IN, H, OUT = 8192, 1024, 2048, 128
NCORES = 8
BS = B // NCORES  # 1024 batch rows per core
P = 128
NF = 512  # matmul moving free dim (fp32 PSUM bank = 512 floats)
NCH = BS // NF  # 2 batch chunks per core

_BUILD_CACHE = {}


def _build_bass():
    import concourse.mybir as mybir
    from concourse import bacc
    from concourse.tile import TileContext

    f16 = mybir.dt.float16
    f32 = mybir.dt.float32

    # Bacc (not raw Bass): its lowering splits multi-sem waits into separate
    # sequencer ops — walrus DMA descriptors only hold one sync wait.
    nc = bacc.Bacc()

    # DRAM I/O (per-core shapes; host pre-tiled)
    # x.T pre-tiled on host into 4 contiguous 512KB blocks: g = ch*2 + j
    # holds strips 4j..4j+3 of batch-chunk ch, laid out [P, 4*NF].
    # Per-queue DMA throughput is size/(size/358GB/s + ~2us completion
    # latency), so 4x512KB beats 16x128KB by ~5us on the critical head.
    xtq = nc.dram_tensor("xtq", [2 * NCH, P, 4 * NF], f16, kind="ExternalInput")
    w0t = nc.dram_tensor("w0t", [H // P, P, (IN // P) * P], f16, kind="ExternalInput")
    w1t = nc.dram_tensor("w1t", [H // P, P, (H // P) * P], f16, kind="ExternalInput")
    w2t = nc.dram_tensor("w2t", [H // P, P, (H // P) * P], f16, kind="ExternalInput")
    w3t = nc.dram_tensor("w3t", [H // P, P, (H // P) * P], f16, kind="ExternalInput")
    betat = nc.dram_tensor("betat", [1, P, (H // P) * P], f16, kind="ExternalInput")
    # fp16 output (host upcasts): halves the final store, well within error budget
    outt = nc.dram_tensor("outt", [OUT, BS], f16, kind="ExternalOutput")

    relu_t = mybir.ActivationFunctionType.Relu

    with TileContext(nc) as tc:
        with (
            tc.tile_pool(name="acts", bufs=1) as acts,
            tc.tile_pool(name="w0pool", bufs=1) as w0pool,
            tc.tile_pool(name="wpool", bufs=8) as wpool,
            tc.tile_pool(name="pp", bufs=3, space="PSUM") as pp,
            tc.tile_pool(name="outp", bufs=1) as outp,
        ):
            # Input shard x.T: 4 tiles of [P, 4*NF] (chunk-major), so layer-0
            # phase A only needs the first 1 MB of x in two transfers
            xq_tiles = [
                acts.tile([P, 4 * NF], f16, tag=f"xq{g}", name=f"xq{g}")
                for g in range(2 * NCH)
            ]
            act_a = [
                acts.tile([P, BS], f16, tag=f"aa{k}", name=f"aa{k}")
                for k in range(H // P)
            ]
            act_b = [
                acts.tile([P, BS], f16, tag=f"ab{k}", name=f"ab{k}")
                for k in range(H // P)
            ]
            out_sb = outp.tile([P, BS], f16, tag="osb", name="osb")

            # PE warm-up: back-to-back tiny matmuls from preamble exit (~7.0us)
            # until the first real operands land (~11.2us). Keeping the PE
            # continuously busy walks the HAM clock-gate through its 3.4us
            # activity window, so the real stream starts at 2.4 GHz instead
            # of paying ~3.5us of half-rate cold matmuls (measured).
            warm_sb = acts.tile([P, P], f16, tag="warm", name="warm_sb")
            # DVE memset: the Pool queue exits the preamble last; DVE lets the
            # warm-up matmuls start ~0.5 us earlier
            nc.vector.memset(warm_sb, 0.0)
            warm_ps = pp.tile([P, P], f32, tag="warm_ps", name="warm_ps", bufs=1)
            for _ in range(38):
                nc.tensor.matmul(warm_ps, warm_sb, warm_sb, start=True, stop=True)

            # x quarter-loads: ACT takes the first (phase-A kt0-3 gate),
            # Pool the second. The phase-B quarters are issued LATER, at the
            # tail of SP's strip chain — their 1MB otherwise shares HBM with
            # the critical xq1/strip transfers inside the 8-16us window and
            # delays xq1 (the dominant head stall) by ~1us.
            nc.scalar.dma_start(xq_tiles[0], xtq[0])
            nc.gpsimd.dma_start(xq_tiles[1], xtq[1])

            # Weight DMAs round-robin over the three DMA paths (SP-HWDGE,
            # Pool-SWDGE, ACT-HWDGE); w0 strips live in their own 16-deep
            # pool because both layer-0 phases read them. Early strips are
            # hand-placed on SP/ACT in need-order (~2.7us per 256KB queue
            # cycle vs one strip consumed per 1.73us of phase-A matmuls).
            dma_engines = [nc.sync, nc.gpsimd, nc.scalar]
            strip_idx = 0
            w0_eng = {0: nc.sync, 1: nc.sync, 2: nc.scalar, 3: nc.sync,
                      4: nc.scalar, 5: nc.sync, 6: nc.scalar, 7: nc.sync}

            # --- Layer 0, two batch-chunk phases ---------------------------
            # Phase ch consumes only x chunk ch, so the matmul stream starts
            # as soon as the first x quarter + w0 strip 0 land (~11us).
            w0_tiles = []
            for mo in range(H // P):
                w0tile = w0pool.tile(
                    [P, (IN // P) * P], f16, tag=f"w0_{mo}", name=f"w0_{mo}"
                )
                if mo in w0_eng:
                    eng = w0_eng[mo]
                else:
                    eng = dma_engines[strip_idx % 3]
                    strip_idx += 1
                eng.dma_start(w0tile, w0t[mo])
                w0_tiles.append(w0tile)

            # phase-B x quarters: behind SP's w0 strips, done ~34us < ~40us need
            nc.sync.dma_start(xq_tiles[2], xtq[2])
            nc.sync.dma_start(xq_tiles[3], xtq[3])

            def l0_relu(mo, ch, ps):
                dst = act_a[mo][:, ch * NF : (ch + 1) * NF]
                if mo % 3 == 2:
                    nc.scalar.activation(dst, ps, relu_t)
                else:
                    nc.vector.tensor_scalar_max(dst, ps, 0.0)

            def l0_mm(ps, mo, ch, kt):
                nc.tensor.matmul(
                    ps,
                    w0_tiles[mo][:, kt * P : (kt + 1) * P],
                    xq_tiles[ch * 2 + kt // 4][:, (kt % 4) * NF : (kt % 4 + 1) * NF],
                    start=(kt == 0),
                    stop=(kt == IN // P - 1),
                )

            for ch in range(NCH):
                if ch == 0:
                    # With the phase-B quarters deferred (v9), xq1 lands
                    # ~13.3-14.2us; interleaving the first 2 groups' K-halves
                    # (kt0-3 of both, then kt4-7) hides that wait behind
                    # ready work while keeping the residual PE idle under the
                    # ~1.7us warm HAM re-throttle threshold.
                    pss = []
                    for mo in range(2):
                        ps = pp.tile(
                            [P, NF], f32, tag=f"ps{mo % 2}",
                            name=f"ps0_0_{mo}", bufs=4 - (mo % 2),
                        )
                        pss.append(ps)
                        for kt in range(IN // P // 2):
                            l0_mm(ps, mo, 0, kt)
                    for mo in range(2):
                        for kt in range(IN // P // 2, IN // P):
                            l0_mm(pss[mo], mo, 0, kt)
                        l0_relu(mo, 0, pss[mo])
                    rest = range(2, H // P)
                else:
                    rest = range(H // P)
                for mo in rest:
                    ps = pp.tile(
                        [P, NF], f32, tag=f"ps{mo % 2}",
                        name=f"ps0_{ch}_{mo}", bufs=4 - (mo % 2),
                    )
                    for kt in range(IN // P):
                        l0_mm(ps, mo, ch, kt)
                    l0_relu(mo, ch, ps)

            # --- Layers 1-3 ------------------------------------------------
            layers = [
                (1, w1t, act_a, act_b),
                (2, w2t, act_b, act_a),
                (3, w3t, act_a, act_b),
            ]
            for li, wd, a_in, a_out in layers:
                kt_n = H // P
                for mo in range(H // P):
                    wtile = wpool.tile(
                        [P, kt_n * P], f16, tag="w", name=f"w{li}_{mo}"
                    )
                    eng = dma_engines[strip_idx % 3]
                    strip_idx += 1
                    eng.dma_start(wtile, wd[mo])
                    # ps0 gets the 8th (otherwise free) PSUM bank
                    pts = [
                        pp.tile(
                            [P, NF], f32, tag=f"ps{no}",
                            name=f"ps{li}_{mo}_{no}", bufs=4 - no,
                        )
                        for no in range(NCH)
                    ]
                    for kt in range(kt_n):
                        lhsT = wtile[:, kt * P : (kt + 1) * P]
                        for no in range(NCH):
                            nc.tensor.matmul(
                                pts[no],
                                lhsT,
                                a_in[kt][:, no * NF : (no + 1) * NF],
                                start=(kt == 0),
                                stop=(kt == kt_n - 1),
                            )
                    # fused relu: PSUM fp32 -> SBUF fp16; alternate DVE/ACT
                    for no in range(NCH):
                        dst = a_out[mo][:, no * NF : (no + 1) * NF]
                        if mo % 3 == 2:
                            nc.scalar.activation(dst, pts[no], relu_t)
                        else:
                            nc.vector.tensor_scalar_max(dst, pts[no], 0.0)

            # Readout: out.T[128, BS] = beta.T @ y3.T (scale folded into beta).
            # Chunk-outer over 4 batch chunks of 256 so each chunk's fp16 copy
            # + store pipelines under the next chunk's matmuls; only the last
            # 64KB store sits on the critical tail.
            btile = wpool.tile([P, (H // P) * P], f16, tag="w", name="btile")
            nc.sync.dma_start(btile, betat[0])
            # Descending chunk sizes: same total PE cycles as 4x256, but the
            # last chunk's copy+store (the only ones on the critical tail)
            # shrink to 128 cols. DVE (CAST) is slightly faster than ACT for
            # the PSUM->fp16 copy, so it gets the last chunk too.
            ro_sizes = [320, 320, 256, 128]
            copy_eng = ["v", "s", "v", "v"]
            store_eng = [nc.scalar, nc.sync, nc.scalar, nc.sync]
            off = 0
            for c, nro in enumerate(ro_sizes):
                psr = pp.tile(
                    [P, nro], f32, tag=f"ps{c % 2}", name=f"ro_{c}", bufs=4 - (c % 2)
                )
                for kt in range(H // P):
                    nc.tensor.matmul(
                        psr,
                        btile[:, kt * P : (kt + 1) * P],
                        act_b[kt][:, off : off + nro],
                        start=(kt == 0),
                        stop=(kt == H // P - 1),
                    )
                dst = out_sb[:, off : off + nro]
                if copy_eng[c] == "v":
                    nc.vector.tensor_copy(dst, psr)
                else:
                    nc.scalar.copy(dst, psr)
                store_eng[c].dma_start(outt[:, off : off + nro], dst)
                off += nro

    nc.finalize()  # runs Bacc passes (incl. multi-wait splitting); PJRT asserts it
    return nc


def _prep_inputs(x, w0, w1, w2, w3, beta):
    """Host-side layout prep: fold scales, transpose, tile, cast to fp16."""

    def tile_weight(w, scale):
        # w: [out, in] fp32 -> wt [in, out] scaled -> [MT, P, KT*P] fp16
        wt = (w.T * scale).astype(np.float16)  # [K, M]
        K, M = wt.shape
        kt_n, mt_n = K // P, M // P
        return np.ascontiguousarray(
            wt.reshape(kt_n, P, mt_n, P).transpose(2, 1, 0, 3).reshape(mt_n, P, kt_n * P)
        )

    w0t = tile_weight(w0, 1.0 / np.sqrt(IN))
    s = 1.0 / np.sqrt(H)
    w1t = tile_weight(w1, s)
    w2t = tile_weight(w2, s)
    w3t = tile_weight(w3, s)
    betat = tile_weight(beta.T, 1.0 / H)  # beta [H, OUT] -> beta.T [OUT, H] "w" form

    x16 = x.astype(np.float16)
    in_maps = []
    for c in range(NCORES):
        xT = x16[c * BS : (c + 1) * BS].T  # [IN, BS]
        # [ch*2+j, p, i*NF+b] = xT[(4j+i)*P+p, ch*NF+b]: 4 contiguous 512KB
        # quarter-blocks, chunk-major (phase A = blocks 0-1)
        xtq = np.ascontiguousarray(
            xT.reshape(2, 4, P, NCH, NF)
            .transpose(3, 0, 2, 1, 4)
            .reshape(2 * NCH, P, 4 * NF)
        )
        in_maps.append(
            {"xtq": xtq, "w0t": w0t, "w1t": w1t, "w2t": w2t, "w3t": w3t, "betat": betat}
        )
    return in_maps


def _run(inputs, trace=False):
    from concourse.bass_utils import run_bass_kernel_spmd

    if "nc" not in _BUILD_CACHE:
        _BUILD_CACHE["nc"] = _build_bass()
    nc = _BUILD_CACHE["nc"]

    in_maps = _prep_inputs(
        np.asarray(inputs["x"], dtype=np.float32),
        np.asarray(inputs["w0"], dtype=np.float32),
        np.asarray(inputs["w1"], dtype=np.float32),
        np.asarray(inputs["w2"], dtype=np.float32),
        np.asarray(inputs["w3"], dtype=np.float32),
        np.asarray(inputs["beta"], dtype=np.float32),
    )

    # The first execution of a freshly-compiled NEFF runs ~20% slower
    # (~500us vs ~415us, device-side cold state that persists across
    # processes once warmed) and occasionally dies with
    # NRT_EXEC_UNIT_UNRECOVERABLE. A throwaway warm-up execution fixes both.
    try:
        run_bass_kernel_spmd(nc, in_maps, core_ids=list(range(NCORES)), trace=False)
    except Exception:  # noqa: BLE001
        pass

    last_err = None
    for attempt in range(3):
        try:
            res = run_bass_kernel_spmd(
                nc, in_maps, core_ids=list(range(NCORES)), trace=trace
            )
            break
        except Exception as e:  # noqa: BLE001
            last_err = e
            import time as _time

            _time.sleep(2.0)
    else:
        raise last_err

    out = np.empty((B, OUT), dtype=np.float32)
    for c in range(NCORES):
        out[c * BS : (c + 1) * BS] = np.asarray(res.results[c]["outt"]).T
    return out, res


def kernel(**inputs):
    out, _ = _run(inputs, trace=False)
    return out



# revision 29
# speedup vs baseline: 1.0077x; 1.0077x over previous
"""Trainium2 Bass kernel for nn_FCN dense MLP.

Reference computation (all fp32):
    y = x                                  # [8192, 1024]
    for w in (w0, w1, w2, w3):             # w: [out, in]
        y = relu((y @ w.T) / sqrt(in))
    out = (y @ beta) / 2048                # beta: [2048, 128] -> [8192, 128]

Strategy:
  - Data-parallel: shard batch 8192 -> 8 cores x 1024 rows. No collectives.
  - Host-side prep (free, not on HW critical path):
      * fold 1/sqrt(in) into each weight, 1/H into beta
      * transpose weights to [in, out], pre-tile to [MT, 128, KT*128] so each
        per-core DMA strip is fully contiguous
      * cast x and weights to fp16 (PE upconverts to FP22, accumulates fp32;
        fp16 keeps 11 mantissa bits -> ~1e-3 relative error)
      * transpose x shard to feature-major [IN, BS]
  - On chip, activations stay feature-major [feature, batch] in SBUF so each
    layer's PSUM output tile [out_feat 128, batch 512] feeds the next layer
    directly as the moving operand (no transposes anywhere on-chip).
  - matmul: lhsT = weight tile [K=128 in-feat, M=128 out-feat] (stationary,
    fp16 -> FWL fast weight load), rhs = act tile [K=128, N=512] (moving,
    fp16 -> 1 cycle/row). PSUM fp32 accumulation over K tiles.
  - ReLU fused into the PSUM->SBUF copy (DVE / ACT alternating), output fp16.
  - Head/tail tuned from trace: 6 warm-up MMs (not 80); layer 0 runs as two
    batch-chunk phases so its matmul stream starts after ~0.25 MB of DMA
    instead of the full 2 MB x load (which is HBM-bandwidth-bound ~6us);
    readout in 4 N=256 chunks with fp16 stores pipelined under the final
    matmuls (host upcasts to fp32).
"""

import sys

if "/opt/trn_rl_repo" not in sys.path:
    sys.path.insert(0, "/opt/trn_rl_repo")

import numpy as np

B, IN, H, OUT = 8192, 1024, 2048, 128
NCORES = 8
BS = B // NCORES  # 1024 batch rows per core
P = 128
NF = 512  # matmul moving free dim (fp32 PSUM bank = 512 floats)
NCH = BS // NF  # 2 batch chunks per core

_BUILD_CACHE = {}


def _build_bass():
    import concourse.mybir as mybir
    from concourse import bacc
    from concourse.tile import TileContext

    f16 = mybir.dt.float16
    f32 = mybir.dt.float32

    # Bacc (not raw Bass): its lowering splits multi-sem waits into separate
    # sequencer ops — walrus DMA descriptors only hold one sync wait.
    nc = bacc.Bacc()

    # DRAM I/O (per-core shapes; host pre-tiled)
    # x.T pre-tiled on host into 4 contiguous 512KB blocks: g = ch*2 + j
    # holds strips 4j..4j+3 of batch-chunk ch, laid out [P, 4*NF].
    # Per-queue DMA throughput is size/(size/358GB/s + ~2us completion
    # latency), so 4x512KB beats 16x128KB by ~5us on the critical head.
    xtq = nc.dram_tensor("xtq", [2 * NCH, P, 4 * NF], f16, kind="ExternalInput")
    w0t = nc.dram_tensor("w0t", [H // P, P, (IN // P) * P], f16, kind="ExternalInput")
    w1t = nc.dram_tensor("w1t", [H // P, P, (H // P) * P], f16, kind="ExternalInput")
    w2t = nc.dram_tensor("w2t", [H // P, P, (H // P) * P], f16, kind="ExternalInput")
    w3t = nc.dram_tensor("w3t", [H // P, P, (H // P) * P], f16, kind="ExternalInput")
    betat = nc.dram_tensor("betat", [1, P, (H // P) * P], f16, kind="ExternalInput")
    # fp16 output (host upcasts): halves the final store, well within error budget
    outt = nc.dram_tensor("outt", [OUT, BS], f16, kind="ExternalOutput")

    relu_t = mybir.ActivationFunctionType.Relu

    with TileContext(nc) as tc:
        with (
            tc.tile_pool(name="acts", bufs=1) as acts,
            tc.tile_pool(name="w0pool", bufs=1) as w0pool,
            tc.tile_pool(name="wpool", bufs=8) as wpool,
            tc.tile_pool(name="pp", bufs=3, space="PSUM") as pp,
            tc.tile_pool(name="outp", bufs=1) as outp,
        ):
            # Input shard x.T: 4 tiles of [P, 4*NF] (chunk-major), so layer-0
            # phase A only needs the first 1 MB of x in two transfers
            xq_tiles = [
                acts.tile([P, 4 * NF], f16, tag=f"xq{g}", name=f"xq{g}")
                for g in range(2 * NCH)
            ]
            act_a = [
                acts.tile([P, BS], f16, tag=f"aa{k}", name=f"aa{k}")
                for k in range(H // P)
            ]
            act_b = [
                acts.tile([P, BS], f16, tag=f"ab{k}", name=f"ab{k}")
                for k in range(H // P)
            ]
            out_sb = outp.tile([P, BS], f16, tag="osb", name="osb")

            # PE warm-up: back-to-back tiny matmuls from preamble exit (~7.0us)
            # until the first real operands land (~11.2us). Keeping the PE
            # continuously busy walks the HAM clock-gate through its 3.4us
            # activity window, so the real stream starts at 2.4 GHz instead
            # of paying ~3.5us of half-rate cold matmuls (measured).
            warm_sb = acts.tile([P, P], f16, tag="warm", name="warm_sb")
            # DVE memset: the Pool queue exits the preamble last; DVE lets the
            # warm-up matmuls start ~0.5 us earlier
            nc.vector.memset(warm_sb, 0.0)
            warm_ps = pp.tile([P, P], f32, tag="warm_ps", name="warm_ps", bufs=1)
            for _ in range(38):
                nc.tensor.matmul(warm_ps, warm_sb, warm_sb, start=True, stop=True)

            # x quarter-loads: ACT takes the first (phase-A kt0-3 gate),
            # Pool the second. The phase-B quarters are issued LATER, at the
            # tail of SP's strip chain — their 1MB otherwise shares HBM with
            # the critical xq1/strip transfers inside the 8-16us window and
            # delays xq1 (the dominant head stall) by ~1us.
            nc.scalar.dma_start(xq_tiles[0], xtq[0])
            nc.gpsimd.dma_start(xq_tiles[1], xtq[1])

            # Weight DMAs round-robin over the three DMA paths (SP-HWDGE,
            # Pool-SWDGE, ACT-HWDGE); w0 strips live in their own 16-deep
            # pool because both layer-0 phases read them. Early strips are
            # hand-placed on SP/ACT in need-order (~2.7us per 256KB queue
            # cycle vs one strip consumed per 1.73us of phase-A matmuls).
            dma_engines = [nc.sync, nc.gpsimd, nc.scalar]
            strip_idx = 0
            w0_eng = {0: nc.sync, 1: nc.sync, 2: nc.scalar, 3: nc.sync,
                      4: nc.scalar, 5: nc.sync, 6: nc.scalar, 7: nc.sync}

            # --- Layer 0, two batch-chunk phases ---------------------------
            # Phase ch consumes only x chunk ch, so the matmul stream starts
            # as soon as the first x quarter + w0 strip 0 land (~11us).
            w0_tiles = []
            for mo in range(H // P):
                w0tile = w0pool.tile(
                    [P, (IN // P) * P], f16, tag=f"w0_{mo}", name=f"w0_{mo}"
                )
                if mo in w0_eng:
                    eng = w0_eng[mo]
                else:
                    eng = dma_engines[strip_idx % 3]
                    strip_idx += 1
                eng.dma_start(w0tile, w0t[mo])
                w0_tiles.append(w0tile)

            # phase-B x quarters: behind SP's w0 strips, done ~34us < ~40us need
            nc.sync.dma_start(xq_tiles[2], xtq[2])
            nc.sync.dma_start(xq_tiles[3], xtq[3])

            for ch in range(NCH):
                for mo in range(H // P):
                    ps = pp.tile(
                        [P, NF], f32, tag=f"ps{mo % 2}",
                        name=f"ps0_{ch}_{mo}", bufs=4 - (mo % 2),
                    )
                    for kt in range(IN // P):
                        nc.tensor.matmul(
                            ps,
                            w0_tiles[mo][:, kt * P : (kt + 1) * P],
                            xq_tiles[ch * 2 + kt // 4][
                                :, (kt % 4) * NF : (kt % 4 + 1) * NF
                            ],
                            start=(kt == 0),
                            stop=(kt == IN // P - 1),
                        )
                    dst = act_a[mo][:, ch * NF : (ch + 1) * NF]
                    if mo % 3 == 2:
                        nc.scalar.activation(dst, ps, relu_t)
                    else:
                        nc.vector.tensor_scalar_max(dst, ps, 0.0)

            # --- Layers 1-3 ------------------------------------------------
            layers = [
                (1, w1t, act_a, act_b),
                (2, w2t, act_b, act_a),
                (3, w3t, act_a, act_b),
            ]
            for li, wd, a_in, a_out in layers:
                kt_n = H // P
                for mo in range(H // P):
                    wtile = wpool.tile(
                        [P, kt_n * P], f16, tag="w", name=f"w{li}_{mo}"
                    )
                    eng = dma_engines[strip_idx % 3]
                    strip_idx += 1
                    eng.dma_start(wtile, wd[mo])
                    # ps0 gets the 8th (otherwise free) PSUM bank
                    pts = [
                        pp.tile(
                            [P, NF], f32, tag=f"ps{no}",
                            name=f"ps{li}_{mo}_{no}", bufs=4 - no,
                        )
                        for no in range(NCH)
                    ]
                    for kt in range(kt_n):
                        lhsT = wtile[:, kt * P : (kt + 1) * P]
                        for no in range(NCH):
                            nc.tensor.matmul(
                                pts[no],
                                lhsT,
                                a_in[kt][:, no * NF : (no + 1) * NF],
                                start=(kt == 0),
                                stop=(kt == kt_n - 1),
                            )
                    # fused relu: PSUM fp32 -> SBUF fp16; alternate DVE/ACT
                    for no in range(NCH):
                        dst = a_out[mo][:, no * NF : (no + 1) * NF]
                        if mo % 3 == 2:
                            nc.scalar.activation(dst, pts[no], relu_t)
                        else:
                            nc.vector.tensor_scalar_max(dst, pts[no], 0.0)

            # Readout: out.T[128, BS] = beta.T @ y3.T (scale folded into beta).
            # Chunk-outer over 4 batch chunks of 256 so each chunk's fp16 copy
            # + store pipelines under the next chunk's matmuls; only the last
            # 64KB store sits on the critical tail.
            btile = wpool.tile([P, (H // P) * P], f16, tag="w", name="btile")
            nc.sync.dma_start(btile, betat[0])
            # Descending chunk sizes: same total PE cycles as 4x256, but the
            # last chunk's copy+store (the only ones on the critical tail)
            # shrink to 128 cols. DVE (CAST) is slightly faster than ACT for
            # the PSUM->fp16 copy, so it gets the last chunk too.
            ro_sizes = [320, 320, 256, 128]
            copy_eng = ["v", "s", "v", "v"]
            store_eng = [nc.scalar, nc.sync, nc.scalar, nc.sync]
            off = 0
            for c, nro in enumerate(ro_sizes):
                psr = pp.tile(
                    [P, nro], f32, tag=f"ps{c % 2}", name=f"ro_{c}", bufs=4 - (c % 2)
                )
                for kt in range(H // P):
                    nc.tensor.matmul(
                        psr,
                        btile[:, kt * P : (kt + 1) * P],
                        act_b[kt][:, off : off + nro],
                        start=(kt == 0),
                        stop=(kt == H // P - 1),
                    )
                dst = out_sb[:, off : off + nro]
                if copy_eng[c] == "v":
                    nc.vector.tensor_copy(dst, psr)
                else:
                    nc.scalar.copy(dst, psr)
                store_eng[c].dma_start(outt[:, off : off + nro], dst)
                off += nro

    nc.finalize()  # runs Bacc passes (incl. multi-wait splitting); PJRT asserts it
    return nc


def _prep_inputs(x, w0, w1, w2, w3, beta):
    """Host-side layout prep: fold scales, transpose, tile, cast to fp16."""

    def tile_weight(w, scale):
        # w: [out, in] fp32 -> wt [in, out] scaled -> [MT, P, KT*P] fp16
        wt = (w.T * scale).astype(np.float16)  # [K, M]
        K, M = wt.shape
        kt_n, mt_n = K // P, M // P
        return np.ascontiguousarray(
            wt.reshape(kt_n, P, mt_n, P).transpose(2, 1, 0, 3).reshape(mt_n, P, kt_n * P)
        )

    w0t = tile_weight(w0, 1.0 / np.sqrt(IN))
    s = 1.0 / np.sqrt(H)
    w1t = tile_weight(w1, s)
    w2t = tile_weight(w2, s)
    w3t = tile_weight(w3, s)
    betat = tile_weight(beta.T, 1.0 / H)  # beta [H, OUT] -> beta.T [OUT, H] "w" form

    x16 = x.astype(np.float16)
    in_maps = []
    for c in range(NCORES):
        xT = x16[c * BS : (c + 1) * BS].T  # [IN, BS]
        # [ch*2+j, p, i*NF+b] = xT[(4j+i)*P+p, ch*NF+b]: 4 contiguous 512KB
        # quarter-blocks, chunk-major (phase A = blocks 0-1)
        xtq = np.ascontiguousarray(
            xT.reshape(2, 4, P, NCH, NF)
            .transpose(3, 0, 2, 1, 4)
            .reshape(2 * NCH, P, 4 * NF)
        )
        in_maps.append(
            {"xtq": xtq, "w0t": w0t, "w1t": w1t, "w2t": w2t, "w3t": w3t, "betat": betat}
        )
    return in_maps


def _run(inputs, trace=False):
    from concourse.bass_utils import run_bass_kernel_spmd

    if "nc" not in _BUILD_CACHE:
        _BUILD_CACHE["nc"] = _build_bass()
    nc = _BUILD_CACHE["nc"]

    in_maps = _prep_inputs(
        np.asarray(inputs["x"], dtype=np.float32),
        np.asarray(inputs["w0"], dtype=np.float32),
        np.asarray(inputs["w1"], dtype=np.float32),
        np.asarray(inputs["w2"], dtype=np.float32),
        np.asarray(inputs["w3"], dtype=np.float32),
        np.asarray(inputs["beta"], dtype=np.float32),
    )

    # The first execution of a freshly-compiled NEFF runs ~20% slower
    # (~500us vs ~415us, device-side cold state that persists across
    # processes once warmed) and occasionally dies with
    # NRT_EXEC_UNIT_UNRECOVERABLE. A throwaway warm-up execution fixes both.
    try:
        run_bass_kernel_spmd(nc, in_maps, core_ids=list(range(NCORES)), trace=False)
    except Exception:  # noqa: BLE001
        pass

    last_err = None
    for attempt in range(3):
        try:
            res = run_bass_kernel_spmd(
                nc, in_maps, core_ids=list(range(NCORES)), trace=trace
            )
            break
        except Exception as e:  # noqa: BLE001
            last_err = e
            import time as _time

            _time.sleep(2.0)
    else:
        raise last_err

    out = np.empty((B, OUT), dtype=np.float32)
    for c in range(NCORES):
        out[c * BS : (c + 1) * BS] = np.asarray(res.results[c]["outt"]).T
    return out, res


def kernel(**inputs):
    out, _ = _run(inputs, trace=False)
    return out

